# revision 22
# baseline (speedup 1.0000x reference)
"""Trainium2 Bass kernel for nn_AttentionBlock (sliding-window GQA attention block).

Sharding: sequence-parallel over 8 cores. Core c owns query rows
[c*512, (c+1)*512) and recomputes K/V for the 3 aligned 512-row blocks
[(c-2)*512, (c+1)*512) that its 1024-wide causal window can touch
(out-of-range blocks are zero-padded and masked).

v2 schedule (engine-balance rework of the phased baseline):
  - K/V phase as before, but with weights on the scalar-engine DMA ring,
    activations on sync, a small first granule so the PE starts earlier,
    fused 3-op RoPE (host tables pre-pack the +/- trig pairs) and
    PE transposes batched 4-at-a-time into one PSUM tile per copy.
  - The old Q phase is dissolved: Q projection blocks are emitted as PE
    filler inside the attention loop (dt=0 interleaves with the first
    group's QK chunks, dt=g+1 rides group g), so the scalar engine's exp
    stream -- the real bottleneck of the attention phase -- hides behind
    projection matmuls.
  - Attention runs per (group, head-pair): exp batches 2 heads per
    activation (heads of a group share K and hence the per-partition
    rstd_k exp scale), halving the 352-cycle ACT overhead vs per-head,
    and the exp'd probabilities live in a compact windowed layout
    (18 KB/partition per head-pair) so two buffers pipeline.
  - PV keeps the P-stationary trick (softmax denominator rides the ones
    column of [V|1]); normalization reads o_ps straight out of PSUM.
  - xq/wq/wo prefetch on the gpsimd/scalar rings during earlier phases;
    y stores alternate sync/gpsimd.
"""

import os
import sys
from contextlib import ExitStack

import numpy as np

for _p in ("/opt/trn_rl_repo",):
    if _p not in sys.path and os.path.isdir(_p):
        sys.path.insert(0, _p)

import concourse.bass as bass
import concourse.mybir as mybir
import concourse.tile as tile
from concourse import bacc
from concourse.bass_utils import run_bass_kernel_spmd
from concourse.masks import make_identity

F16 = mybir.dt.float16
F32 = mybir.dt.float32

N_CORES = 8
S, D = 4096, 2048
H, KV, DK = 16, 4, 128
GSZ = H // KV  # heads per kv group
WINDOW = 1024
THETA = 500000.0
EPS = 1e-6

SQ = S // N_CORES          # 512 query rows per core
NQT = SQ // 128            # 4 query chunks
NKT = 12                   # 12 kv chunks of 128 (3 blocks of 512)
SKV = NKT * 128            # 1536
NE = D // 128              # 16 contraction chunks
NDT = D // 512             # 4 tiles of 512 along output dims

KV_GRAN = [1, 3, 4, 4, 4]  # ec granules per staged block (small first DMA)

# compact windowed layout of the exp'd scores, per head-pair:
# for each kv chunk kc the valid query span is [lo, hi); the two heads'
# [hi-lo] rows are stored back to back at base PBL[kc].
PBL = []
_off = 0
for _kc in range(NKT):
    _lo = max(0, _kc - 8) * 128
    _hi = (min(NQT - 1, _kc) + 1) * 128
    PBL.append((_off, _lo, _hi))
    _off += 2 * (_hi - _lo)
PTOT = _off  # 9216 f16 elements per partition


def _broadcast_free(ap, count, axis):
    """Insert a 0-step (broadcast) free dim of length `count` at `axis`
    (free-dim index, 0-based after the partition dim)."""
    new = list(ap.ap)
    new.insert(1 + axis, [0, count])
    return bass.AP(tensor=ap.tensor, offset=ap.offset, ap=new)


def _stage_chunks(nc, pool, eng, src, granules, tag):
    """DMA src [128, NE, 512] into tiles of `granules` ec-chunks on `eng`'s
    queue; returns (tiles, ecmap) with ecmap[ec] = (tile_idx, local_idx)."""
    tiles, ecmap = [], []
    e0 = 0
    for gi, gcnt in enumerate(granules):
        t = pool.tile([128, gcnt, 512], F16, tag=f"{tag}{gi}", name=f"{tag}{gi}")
        eng.dma_start(out=t, in_=src[:, e0:e0 + gcnt, :])
        tiles.append(t)
        for j in range(gcnt):
            ecmap.append((gi, j))
        e0 += gcnt
    return tiles, ecmap


def _emit_rope(nc, pool, src, dst, tab, nheads, cast_scalars=None):
    """dst = rope(src) for nheads 128-wide head slices, 3 fused vector ops.

    tab: [128, 2, 2, 64] AP slice (ev/od major, then the two trig tables
    with signs pre-folded so both combining ops are adds).
    If cast_scalars is given (list of nheads [128,1] APs) the result is
    written per head as dst_head = tmp_head * scalar (fused cast).
    """
    r = src.rearrange("p (h m two) -> p h m two", two=2, m=64)
    ev, od = r[:, :, :, 0], r[:, :, :, 1]
    ev_b = _broadcast_free(ev, 2, 1)   # [p, h, 2, 64]
    od_b = _broadcast_free(od, 2, 1)
    tab_ev = _broadcast_free(tab[:, 0, :, :], nheads, 0)
    tab_od = _broadcast_free(tab[:, 1, :, :], nheads, 0)
    t1 = pool.tile([128, nheads, 2, 64], F32, tag="rope_t1", name="rope_t1")
    t2 = pool.tile([128, nheads, 2, 64], F32, tag="rope_t2", name="rope_t2")
    nc.vector.tensor_mul(t1, ev_b, tab_ev)
    nc.vector.tensor_mul(t2, od_b, tab_od)
    if cast_scalars is None:
        dst_v = dst.rearrange("p (h m two) -> p h two m", two=2, m=64)
        nc.vector.tensor_add(dst_v, t1, t2)
    else:
        tmp = pool.tile([128, nheads * 128], F32, tag="rope_tmp", name="rope_tmp")
        tmp_v = tmp.rearrange("p (h m two) -> p h two m", two=2, m=64)
        nc.vector.tensor_add(tmp_v, t1, t2)
        for hh in range(nheads):
            nc.vector.tensor_scalar_mul(
                dst[:, hh * 128:(hh + 1) * 128],
                tmp[:, hh * 128:(hh + 1) * 128],
                cast_scalars[hh],
            )


def _rms_stats4(nc, pool, src, sqrt_bias, sqrt_scale, out_recip4):
    """out_recip4[128,4] = 1/sqrt(sum(head_sq)*sqrt_scale + sqrt_bias) for the
    four 128-wide head slices of a [128, 512] src tile."""
    ssq4 = pool.tile([128, 4], F32, tag="rms_ssq4", name="rms_ssq4")
    for hh in range(4):
        scr = pool.tile([128, 128], F32, tag="rms_scr", name="rms_scr")
        nc.scalar.activation(out=scr, in_=src[:, hh * 128:(hh + 1) * 128],
                             func=mybir.ActivationFunctionType.Square,
                             accum_out=ssq4[:, hh:hh + 1])
    srt4 = pool.tile([128, 4], F32, tag="rms_srt4", name="rms_srt4")
    nc.scalar.activation(out=srt4, in_=ssq4, func=mybir.ActivationFunctionType.Sqrt,
                         bias=sqrt_bias, scale=sqrt_scale)
    nc.vector.reciprocal(out=out_recip4, in_=srt4)


def build_program():
    nc = bacc.Bacc("TRN2", target_bir_lowering=False, debug=False)

    xq_t = nc.declare_dram_parameter("xq_t", [128, NE, SQ], F16, isOutput=False)
    xk_t = nc.declare_dram_parameter("xk_t", [3, 128, NE, 512], F16, isOutput=False)
    xv_t = nc.declare_dram_parameter("xv_t", [3, 128, NE, 512], F16, isOutput=False)
    wq_t = nc.declare_dram_parameter("wq_t", [NDT, 128, NE, 512], F16, isOutput=False)
    wk_t = nc.declare_dram_parameter("wk_t", [128, NE, 512], F16, isOutput=False)
    wv_t = nc.declare_dram_parameter("wv_t", [128, NE, 512], F16, isOutput=False)
    wo_t = nc.declare_dram_parameter("wo_t", [NDT, 128, NE, 512], F16, isOutput=False)
    ropeq = nc.declare_dram_parameter("ropeq", [128, NQT, 2, 2, 64], F32, isOutput=False)
    ropek = nc.declare_dram_parameter("ropek", [128, NKT, 2, 2, 64], F32, isOutput=False)
    pmask = nc.declare_dram_parameter("pmask", [128, NQT, 2, 128], F16, isOutput=False)
    padcol = nc.declare_dram_parameter("padcol", [128, NQT], F32, isOutput=False)
    y = nc.declare_dram_parameter("y", [SQ, D], F32, isOutput=True)

    EXP = mybir.ActivationFunctionType.Exp

    with tile.TileContext(nc) as tc, ExitStack() as es:
        const = es.enter_context(tc.tile_pool(name="const", bufs=1))
        persist = es.enter_context(tc.tile_pool(name="persist", bufs=1))
        wop = es.enter_context(tc.tile_pool(name="wo_pool", bufs=1))
        qps = es.enter_context(tc.tile_pool(name="q_ps", bufs=1, space="PSUM"))
        tpp = es.enter_context(tc.tile_pool(name="tp_ps", bufs=1, space="PSUM"))

        ident = const.tile([128, 128], F16)
        make_identity(nc, ident)
        bias_k = const.tile([128, 1], F32)
        nc.vector.memset(bias_k, 128.0 * EPS)
        bias_q = const.tile([128, 1], F32)
        nc.vector.memset(bias_q, EPS)
        # const tiles whose DMAs are emitted at the KV->attention bridge
        rq_sb = const.tile([128, NQT, 2, 2, 64], F32)
        masks = const.tile([128, NQT, 2, 128], F16)
        padcol_sb = const.tile([128, NQT], F32)

        kT = persist.tile([128, KV, NKT, 128], F16)
        vt = persist.tile([128, NKT, KV, 132], F16)
        nc.vector.memset(vt[:, :, :, 128:129], 1.0)
        qT = persist.tile([128, H, SQ], F16)
        aoT = persist.tile([128, H, SQ], F16)
        rstdk = persist.tile([128, NKT, KV], F32)

        wo_sb = {}

        def load_wo(ot):
            t = wop.tile([128, NE, 512], F16, tag=f"wo{ot % 2}", name=f"wo{ot}")
            nc.gpsimd.dma_start(out=t, in_=wo_t[ot][:, :, :])
            wo_sb[ot] = t

        # ---------------- K/V phase ----------------
        with tc.tile_pool(name="kv_rk", bufs=2) as rkp, \
             tc.tile_pool(name="kv_w", bufs=1) as kvw, \
             tc.tile_pool(name="kv_stage", bufs=1) as kvs, \
             tc.tile_pool(name="kv_sb", bufs=2) as kvsb, \
             tc.tile_pool(name="kv_ps", bufs=3, space="PSUM") as kvps:
            rk_b = rkp.tile([128, 4, 2, 2, 64], F32, tag="rk", name="rk0")
            nc.gpsimd.dma_start(out=rk_b, in_=ropek[:, 0:4, :, :, :])
            # K path first: first matmul gates on one small granule of
            # xk (sync ring) + wk (scalar ring).
            xk_c, xkmap = _stage_chunks(nc, kvs, nc.sync, xk_t[0], KV_GRAN, "xk")
            wk_c, wkmap = _stage_chunks(nc, kvw, nc.scalar, wk_t, KV_GRAN, "wk")
            xv_c, xvmap = _stage_chunks(nc, kvs, nc.sync, xv_t[0], [4] * 4, "xv")
            wv_c, wvmap = _stage_chunks(nc, kvw, nc.scalar, wv_t, [4] * 4, "wv")
            for b in range(3):
                if b > 0:
                    xk_c, xkmap = _stage_chunks(nc, kvs, nc.sync, xk_t[b],
                                                KV_GRAN, "xk")
                    xv_c, xvmap = _stage_chunks(nc, kvs, nc.sync, xv_t[b],
                                                [4] * 4, "xv")
                    rk_b = rkp.tile([128, 4, 2, 2, 64], F32, tag="rk",
                                    name=f"rk{b}")
                    nc.gpsimd.dma_start(out=rk_b,
                                        in_=ropek[:, 4 * b:4 * b + 4, :, :, :])
                for sc in range(4):
                    kc = b * 4 + sc
                    ssl = slice(sc * 128, (sc + 1) * 128)
                    k_ps = kvps.tile([128, 512], F32, tag="kps")
                    for ec in range(NE):
                        ti, tj = xkmap[ec], wkmap[ec]
                        nc.tensor.matmul(k_ps, xk_c[ti[0]][:, ti[1], ssl],
                                         wk_c[tj[0]][:, tj[1], :],
                                         start=(ec == 0), stop=(ec == NE - 1))
                    _rms_stats4(nc, kvsb, k_ps, sqrt_bias=bias_k,
                                sqrt_scale=1.0, out_recip4=rstdk[:, kc, :])
                    krot = kvsb.tile([128, 512], F16, tag="krot")
                    _emit_rope(nc, kvsb, k_ps[:, :], krot[:, :],
                               rk_b[:, sc], KV)
                    ktp = tpp.tile([128, 4, 128], F16, tag="tp", name="ktp")
                    for g in range(KV):
                        nc.tensor.transpose(ktp[:, g, :],
                                            krot[:, g * 128:(g + 1) * 128], ident)
                    nc.vector.tensor_copy(out=kT[:, :, kc, :], in_=ktp)
                    v_ps = kvps.tile([128, 512], F32, tag="vps")
                    for ec in range(NE):
                        ti, tj = xvmap[ec], wvmap[ec]
                        nc.tensor.matmul(v_ps, xv_c[ti[0]][:, ti[1], ssl],
                                         wv_c[tj[0]][:, tj[1], :],
                                         start=(ec == 0), stop=(ec == NE - 1))
                    nc.vector.tensor_copy(
                        out=vt[:, kc, :, 0:128],
                        in_=v_ps.rearrange("p (g d) -> p g d", g=KV))

        # ---------------- bridge: Q-side loads ----------------
        # kv staging pools just released; these allocations reuse that
        # space and their DMAs run while the PE drains the K/V tail.
        q_es = ExitStack()
        xqp = q_es.enter_context(tc.tile_pool(name="xq_pool", bufs=1,
                                              side="right"))
        wqp = q_es.enter_context(tc.tile_pool(name="wq_pool", bufs=1,
                                              side="right"))
        qsb = q_es.enter_context(tc.tile_pool(name="q_sb", bufs=2,
                                              side="right"))
        nc.gpsimd.dma_start(out=rq_sb, in_=ropeq[:, :, :, :, :])
        nc.gpsimd.dma_start(out=masks, in_=pmask[:, :, :, :])
        nc.gpsimd.dma_start(out=padcol_sb, in_=padcol[:, :])
        xq_sb = xqp.tile([128, NE, SQ], F16)
        nc.gpsimd.dma_start(out=xq_sb, in_=xq_t[:, :, :])
        wq_sb = {}

        def load_wq(dt, eng):
            t = wqp.tile([128, NE, 512], F16, tag="wq", name=f"wq{dt}")
            eng.dma_start(out=t, in_=wq_t[dt][:, :, :])
            wq_sb[dt] = t

        load_wq(0, nc.scalar)

        def emit_qp(dt, sc):
            """Q projection block: 16 MMs + rms + rope + 4 batched
            transposes + one strided copy into qT."""
            ssl = slice(sc * 128, (sc + 1) * 128)
            q_ps = qps.tile([128, 512], F32, tag="qps")
            w = wq_sb[dt]
            for ec in range(NE):
                nc.tensor.matmul(q_ps, xq_sb[:, ec, ssl], w[:, ec, :],
                                 start=(ec == 0), stop=(ec == NE - 1))
            rq4 = qsb.tile([128, 4], F32, tag="rstdq4")
            _rms_stats4(nc, qsb, q_ps, sqrt_bias=bias_q,
                        sqrt_scale=1.0 / 128.0, out_recip4=rq4)
            rstd_q = [rq4[:, hh:hh + 1] for hh in range(4)]
            qrot = qsb.tile([128, 512], F16, tag="qrot")
            _emit_rope(nc, qsb, q_ps[:, :], qrot[:, :], rq_sb[:, sc],
                       4, cast_scalars=rstd_q)
            qtp = tpp.tile([128, 4, 128], F16, tag="tp", name="qtp")
            for hh in range(4):
                nc.tensor.transpose(qtp[:, hh, :],
                                    qrot[:, hh * 128:(hh + 1) * 128], ident)
            nc.vector.tensor_copy(out=qT[:, dt * 4:(dt + 1) * 4, ssl], in_=qtp)

        # ---------------- attention (+ Q filler) ----------------
        with tc.tile_pool(name="p_pool", bufs=2) as pp, \
             tc.tile_pool(name="a_sb", bufs=3) as asb, \
             tc.tile_pool(name="a_sc", bufs=2, space="PSUM") as asc, \
             tc.tile_pool(name="a_oc", bufs=2, space="PSUM") as aoc:
            for g in range(KV):
                for hp in range(2):
                    P2 = pp.tile([128, PTOT], F16, tag="P", name=f"P{g}_{hp}")
                    pbase = P2[:, :]

                    def pap(off_, dims):
                        return bass.AP(tensor=pbase.tensor,
                                       offset=pbase.offset + off_,
                                       ap=[pbase.ap[0]] + dims)

                    for kc in range(NKT):
                        base, lo, hi = PBL[kc]
                        w = hi - lo
                        # Q-projection fillers keep the PE fed while the
                        # scalar engine chews on exp
                        if g == 0 and hp == 0 and kc <= 3:
                            emit_qp(0, kc)
                        s_ps = asc.tile([128, 2, 512], F32, tag="score")
                        for hh in range(2):
                            h = g * GSZ + hp * 2 + hh
                            nc.tensor.matmul(s_ps[:, hh, lo:hi],
                                             kT[:, g, kc, :],
                                             qT[:, h, lo:hi],
                                             start=True, stop=True)
                        nc.scalar.activation(
                            out=pap(base, [[w, 2], [1, w]]),
                            in_=s_ps[:, :, lo:hi], func=EXP,
                            scale=rstdk[:, kc, g:g + 1])
                        # corner masks: window edge (kc=qb) / causal diag
                        for jj, qb in ((0, kc), (1, kc - 8)):
                            if 0 <= qb < NQT:
                                psl = pap(base + (qb * 128 - lo),
                                          [[w, 2], [1, 128]])
                                m = _broadcast_free(masks[:, qb, jj, :], 2, 0)
                                nc.vector.tensor_mul(psl, psl, m)
                        if g < 3 and hp == 1 and kc in (3, 5, 7, 9):
                            emit_qp(g + 1, (kc - 3) // 2)
                        if g < 3 and hp == 0 and kc == 4:
                            load_wq(g + 1, nc.gpsimd)
                        if g == 3 and hp == 0 and kc == 2:
                            load_wo(0)
                        if kc >= 8:
                            qb = kc - 8
                            qbs = slice(qb * 128, (qb + 1) * 128)
                            tp = tpp.tile([128, 2, 128], F16, tag="tp",
                                          name="pvtp")
                            for hh in range(2):
                                o_ps = aoc.tile([128, 132], F32, tag="oacc")
                                for j in range(9):
                                    kcj = qb + j
                                    bj, loj, hij = PBL[kcj]
                                    wj = hij - loj
                                    stat = pap(bj + hh * wj + (qb * 128 - loj),
                                               [[1, 128]])
                                    nc.tensor.matmul(o_ps[:, 0:129], stat,
                                                     vt[:, kcj, g, 0:129],
                                                     start=(j == 0),
                                                     stop=(j == 8))
                                den = asb.tile([128, 1], F32, tag="den")
                                nc.vector.scalar_tensor_tensor(
                                    out=den, in0=o_ps[:, 128:129],
                                    scalar=1.0, in1=padcol_sb[:, qb:qb + 1],
                                    op0=mybir.AluOpType.mult,
                                    op1=mybir.AluOpType.subtract)
                                den_r = asb.tile([128, 1], F32, tag="denr")
                                nc.vector.reciprocal(out=den_r, in_=den)
                                o_nrm = asb.tile([128, 128], F16, tag="onrm")
                                nc.vector.tensor_scalar_mul(
                                    o_nrm, o_ps[:, 0:128], den_r)
                                nc.tensor.transpose(tp[:, hh, :], o_nrm, ident)
                            h0 = g * GSZ + hp * 2
                            nc.vector.tensor_copy(
                                out=aoT[:, h0:h0 + 2, qbs], in_=tp)
                if g == 2:
                    # xq/wq/q-scratch are dead once QP(3) is emitted
                    q_es.close()

        # ---------------- output projection ----------------
        with tc.tile_pool(name="o_sb", bufs=3) as osb, \
             tc.tile_pool(name="o_ps", bufs=3, space="PSUM") as ops:
            load_wo(1)
            for ot in range(NDT):
                w = wo_sb[ot]
                for sc in range(NQT):
                    ssl = slice(sc * 128, (sc + 1) * 128)
                    y_ps = ops.tile([128, 512], F32, tag="yacc")
                    for dc in range(NE):
                        nc.tensor.matmul(y_ps, aoT[:, dc, ssl],
                                         w[:, dc, :],
                                         start=(dc == 0), stop=(dc == NE - 1))
                    y_sb = osb.tile([128, 512], F32, tag="ysb")
                    nc.vector.tensor_copy(out=y_sb, in_=y_ps)
                    eng = nc.sync if (sc % 2 == 0) else nc.gpsimd
                    eng.dma_start(
                        out=y[sc * 128:(sc + 1) * 128, ot * 512:(ot + 1) * 512],
                        in_=y_sb)
                # prefetch two iterations out; emitted after this ot's
                # matmuls so the slot WAR is tracked
                if ot + 2 < NDT:
                    load_wo(ot + 2)

    nc.compile()
    return nc


# ---------------- host-side packing ----------------

def _tile_emajor(a16, col0, ncols):
    """[2048, N] (e-major) f16 array -> [128, 16, ncols] tiled view."""
    sl = a16[:, col0:col0 + ncols]
    return np.ascontiguousarray(sl.reshape(NE, 128, ncols).transpose(1, 0, 2))


def _rope_tables(pos, norm_w):
    """-> [128, nchunks, 2, 2, 64] f32 tables with the per-dim norm weights
    folded in and the signs arranged so the on-chip rope combine is
    two mults + one add:
      [ck, 0, 0] = cos*w_ev   [ck, 0, 1] = sin*w_ev     (terms from x_even)
      [ck, 1, 0] = -sin*w_od  [ck, 1, 1] = cos*w_od     (terms from x_odd)
    pos: [n*128] positions."""
    freqs = 1.0 / (THETA ** (np.arange(0, DK, 2, dtype=np.float64) / DK))
    ang = np.outer(pos.astype(np.float64), freqs)
    cos = np.cos(ang).astype(np.float32)
    sin = np.sin(ang).astype(np.float32)
    w_ev = norm_w[0::2].astype(np.float32)
    w_od = norm_w[1::2].astype(np.float32)
    n = pos.shape[0] // 128
    tabs = np.stack([
        np.stack([cos * w_ev, sin * w_ev], axis=1),       # ev terms
        np.stack([-sin * w_od, cos * w_od], axis=1),      # od terms
    ], axis=1)  # [n*128, 2, 2, 64]
    return np.ascontiguousarray(
        tabs.reshape(n, 128, 2, 2, 64).transpose(1, 0, 2, 3, 4))


def _masks_for_core(c):
    """Corner masks only: jj=0 -> kc=qb (window edge, lag 8);
    jj=1 -> kc=qb+8 (causal diagonal, lag 0)."""
    out = np.zeros((128, NQT, 2, 128), np.float16)
    p = np.arange(128)
    q = np.arange(128)
    for qb in range(NQT):
        for jj, j in ((0, 0), (1, 8)):
            kchunk = c * 4 - 8 + qb + j
            iglob = c * SQ + qb * 128 + q[None, :]
            jglob = kchunk * 128 + p[:, None]
            ok = (jglob >= 0) & (iglob - jglob >= 0) & (iglob - jglob < WINDOW)
            out[:, qb, jj, :] = ok.astype(np.float16)
    return out


def _padcol_for_core(c):
    """Per-query-block count of zero-padded keys on interior (unmasked)
    window tiles: exp(0)=1 each, subtracted from the softmax denominator.
    Interior tiles are kc=qb+1..qb+7; tile kc is fully padded iff global
    chunk c*4-8+kc < 0."""
    out = np.zeros((128, NQT), np.float32)
    for qb in range(NQT):
        npad = int(np.clip(7 - 4 * c - qb, 0, 7))
        out[:, qb] = 128.0 * npad
    return out


_PROGRAM = None


def _get_program():
    global _PROGRAM
    if _PROGRAM is None:
        _PROGRAM = build_program()
    return _PROGRAM


def _pack_in_maps(xq, xk, xv, Wq, Wk, Wv, Wo, q_norm_w, k_norm_w):
    xqT = np.ascontiguousarray(np.asarray(xq, np.float32)[0].T).astype(np.float16)
    xkT = np.asarray(xk, np.float32)[0].T.astype(np.float16)
    xvT = np.asarray(xv, np.float32)[0].T.astype(np.float16)
    pad = np.zeros((D, 2 * SQ), np.float16)
    xkTp = np.concatenate([pad, xkT], axis=1)  # col i = global row i - 1024
    xvTp = np.concatenate([pad, xvT], axis=1)

    wq16 = np.ascontiguousarray(np.asarray(Wq, np.float32).T).astype(np.float16)
    wk16 = np.ascontiguousarray(np.asarray(Wk, np.float32).T).astype(np.float16)
    wv16 = np.ascontiguousarray(np.asarray(Wv, np.float32).T).astype(np.float16)
    wo16 = np.ascontiguousarray(np.asarray(Wo, np.float32).T).astype(np.float16)

    wq_t = np.stack([_tile_emajor(wq16, dt * 512, 512) for dt in range(NDT)])
    wk_t = _tile_emajor(wk16, 0, 512)
    wv_t = _tile_emajor(wv16, 0, 512)
    wo_t = np.stack([_tile_emajor(wo16, ot * 512, 512) for ot in range(NDT)])

    qw = np.asarray(q_norm_w, np.float32)
    kw = np.asarray(k_norm_w, np.float32)

    in_maps = []
    for c in range(N_CORES):
        xq_tc = _tile_emajor(xqT, c * SQ, SQ)
        xk_tc = np.stack([_tile_emajor(xkTp, (c + b) * 512, 512) for b in range(3)])
        xv_tc = np.stack([_tile_emajor(xvTp, (c + b) * 512, 512) for b in range(3)])
        qpos = c * SQ + np.arange(SQ)
        kpos = (c - 2) * 512 + np.arange(SKV)
        in_maps.append({
            "xq_t": xq_tc, "xk_t": xk_tc, "xv_t": xv_tc,
            "wq_t": wq_t, "wk_t": wk_t, "wv_t": wv_t, "wo_t": wo_t,
            "ropeq": _rope_tables(qpos, qw),
            "ropek": _rope_tables(kpos, kw),
            "pmask": _masks_for_core(c),
            "padcol": _padcol_for_core(c),
        })
    return in_maps


def kernel(xq, xk, xv, Wq, Wk, Wv, Wo, q_norm_w, k_norm_w):
    nc = _get_program()
    in_maps = _pack_in_maps(xq, xk, xv, Wq, Wk, Wv, Wo, q_norm_w, k_norm_w)
    res = run_bass_kernel_spmd(nc, in_maps, core_ids=list(range(N_CORES)))
    out = np.concatenate([res.results[c]["y"] for c in range(N_CORES)], axis=0)
    return out.reshape(1, S, D).astype(np.float32)


def kernel_with_results(trace=False, tmpdir=None, **inputs):
    """Devloop entry: same as kernel() but also returns the raw
    BassKernelResults (exec_time_ns etc. when trace is enabled)."""
    nc = _get_program()
    in_maps = _pack_in_maps(**inputs)
    res = run_bass_kernel_spmd(nc, in_maps, core_ids=list(range(N_CORES)),
                               trace=trace, tmpdir=tmpdir)
    out = np.concatenate([res.results[c]["y"] for c in range(N_CORES)], axis=0)
    return out.reshape(1, S, D).astype(np.float32), res
